# revision 2
# baseline (speedup 1.0000x reference)
# Trainium2 Bass kernel for nn_ClusteringLayer (DEC soft-assignment / Student-t
# codebook posterior):
#   d2[n,k] = ||x_n - c_k||^2 ;  q = 1/(1+d2) row-normalized over k  (alpha=1).
#
# Sharding: data-parallel along N over 8 NeuronCores; clusters replicated.
# Per core: x_shard (16384, 512) f32 -> q_shard (16384, 128) f32.
#
# Engine balance per 1024-row super-tile (measured-driven; DVE is the
# critical engine, so work is spread):
#   Pool/SWDGE: cast-load x f32->bf16 (software-pipelined one tile ahead)
#   ACT:        per-tile Square + fp32 accum -> x2
#   PE:         4 transposes/tile -> PSUM; 4 bf16 MMs + rank-1 (1+c2) row
#   DVE:        PSUM->SBUF xT copies (batched FD=2048), u = psum + x2,
#               reciprocal, row-sum, tiny reciprocal, normalize multiply
#   SP/HWDGE:   store q
import dataclasses

import numpy as np

import concourse.bass as bass
import concourse.mybir as mybir
from concourse import bacc
from concourse.bass import ts
from concourse.masks import make_identity
from concourse.tile import TileContext

N, D, K = 131072, 512, 128
N_CORES = 8
NS = N // N_CORES  # rows per core
P = 128  # partitions / row-tile size
G = 8    # row-tiles per super-tile
F32 = mybir.dt.float32
BF16 = mybir.dt.bfloat16


def _bcast_free(ap: bass.AP, n: int) -> bass.AP:
    """Append a step-0 (broadcast) innermost free dim of size n."""
    return dataclasses.replace(ap, ap=list(ap.ap) + [[0, n]])


def build(ns=NS, g=G, repeat=1, dma_mode="normal", xin_bufs=3, xt_bufs=3,
          ep_bufs=3, qo_bufs=3, ps_t_bufs=2, ps_q_bufs=2, qo_bf16=0):
    n_super = ns // (P * g)
    assert ns == n_super * P * g
    n_dchunk = D // P  # 4
    half_g = g // 2

    nc = bacc.Bacc("TRN2", target_bir_lowering=False, debug=False)
    x_dram = nc.dram_tensor("x", [ns, D], F32, kind="ExternalInput")
    c_dram = nc.dram_tensor("clusters", [K, D], F32, kind="ExternalInput")
    q_dram = nc.dram_tensor("q", [ns, K], BF16 if qo_bf16 else F32,
                            kind="ExternalOutput")

    with TileContext(nc) as tc:
        with (
            tc.tile_pool(name="const", bufs=1) as const_pool,
            tc.tile_pool(name="xin", bufs=xin_bufs) as xin_pool,
            tc.tile_pool(name="xt", bufs=xt_bufs) as xt_pool,
            tc.tile_pool(name="ep", bufs=ep_bufs) as ep_pool,
            tc.tile_pool(name="qo", bufs=qo_bufs) as qo_pool,
            tc.tile_pool(name="ps_t", bufs=ps_t_bufs, space="PSUM") as ps_t_pool,
            tc.tile_pool(name="ps_q", bufs=ps_q_bufs, space="PSUM") as ps_q_pool,
        ):
            # ---------------- setup (once) ----------------
            ident_bf = const_pool.tile([P, P], BF16)
            make_identity(nc, ident_bf)

            c_f32 = const_pool.tile([K, D], F32)
            nc.sync.dma_start(c_f32[:], c_dram[:, :])
            c_bf = const_pool.tile([K, D], BF16)
            nc.vector.tensor_copy(c_bf[:], c_f32[:])

            # c2[k] = sum_d c_bf[k,d]^2 (fp32 accum), then 1 + c2 as bf16
            csq = const_pool.tile([K, D], F32)
            c2 = const_pool.tile([K, 1], F32)
            nc.scalar.activation(
                csq[:], c_bf[:], mybir.ActivationFunctionType.Square,
                accum_out=c2[:],
            )
            c2p1_bf = const_pool.tile([K, 1], BF16)
            nc.vector.tensor_scalar_add(c2p1_bf[:], c2[:], 1.0)

            # transpose (1+c2) -> row [1, K] bf16
            ps_row = ps_t_pool.tile([1, K], BF16, tag="ps_xt")
            nc.tensor.transpose(ps_row[:], c2p1_bf[:], ident_bf[:])
            c2p1_row = const_pool.tile([1, K], BF16)
            nc.vector.tensor_copy(c2p1_row[:], ps_row[:])

            ones_row = const_pool.tile([1, K], BF16)
            nc.vector.memset(ones_row[:], 1.0)

            # cTm2[p, c, k] = -2 * clusters_bf[k, c*128+p]
            cTm2 = const_pool.tile([P, n_dchunk, K], BF16)
            for c in range(n_dchunk):
                ps_c = ps_t_pool.tile([P, P], BF16, tag="ps_xt")
                nc.tensor.transpose(ps_c[:], c_bf[:, ts(c, P)], ident_bf[:])
                nc.vector.tensor_scalar_mul(cTm2[:, c, :], ps_c[:], -2.0)

            # ---------------- main loop ----------------
            # x loads software-pipelined: st+1's SWDGE cast-load issued at
            # the start of st's body.
            def issue_load(sti):
                n0 = (sti % n_super) * P * g
                x_view = x_dram[n0:n0 + P * g, :].rearrange(
                    "(gg p) d -> p gg d", p=P)
                t = xin_pool.tile([P, g, D], BF16, name="x_nat", tag="x_nat")
                nc.gpsimd.dma_start(t[:], x_view)
                return t

            n_total = n_super * repeat
            if dma_mode == "once":
                x_once = issue_load(0)
            else:
                pending = issue_load(0)
            for sti in range(n_total):
                st = sti % n_super
                n0 = st * P * g
                if dma_mode == "once":
                    x_nat = x_once
                else:
                    x_nat = pending
                    if sti + 1 < n_total:
                        pending = issue_load(sti + 1)

                x2s = ep_pool.tile([P, g], F32, tag="x2s")
                psum_q = ps_q_pool.tile([P, g, K], F32)

                # squares (ACT) + transposes (PE); copies batched per 4 tiles
                for gp in range(g // half_g):  # 2 groups of 4 row-tiles
                    ps_xt = ps_t_pool.tile(
                        [P, half_g * n_dchunk, P], BF16, tag="ps_xt")
                    for gi in range(half_g):
                        gg = gp * half_g + gi
                        sq_scr = xt_pool.tile([P, D], BF16, tag="sq")
                        nc.scalar.activation(
                            sq_scr[:], x_nat[:, gg, :],
                            mybir.ActivationFunctionType.Square,
                            accum_out=x2s[:, gg:gg + 1],
                        )
                        for c in range(n_dchunk):
                            nc.tensor.transpose(
                                ps_xt[:, gi * n_dchunk + c, :],
                                x_nat[:, gg, ts(c, P)], ident_bf[:])
                    xt2 = xt_pool.tile(
                        [P, half_g * n_dchunk, P], BF16, tag=f"xt{gp}")
                    nc.vector.tensor_copy(xt2[:], ps_xt[:])

                    for gi in range(half_g):
                        gg = gp * half_g + gi
                        for c in range(n_dchunk):
                            nc.tensor.matmul(
                                psum_q[:, gg, :],
                                lhsT=xt2[:, gi * n_dchunk + c, :],
                                rhs=cTm2[:, c, :], start=(c == 0), stop=False)
                        nc.tensor.matmul(
                            psum_q[:, gg, :], lhsT=ones_row[:],
                            rhs=c2p1_row[:], start=False, stop=True)

                # u = psum + x2[n] broadcast along k (clamp at 1.0 skipped:
                # d2 >= -1e-4 numerically; ref's max(d2,0) differs <=1e-5)
                u = ep_pool.tile([P, g, K], F32, tag="u")
                nc.vector.tensor_tensor(
                    out=u[:], in0=psum_q[:],
                    in1=_bcast_free(x2s[:], K),
                    op=mybir.AluOpType.add,
                )
                qun = ep_pool.tile([P, g, K], F32, tag="qun")
                nc.vector.reciprocal(qun[:], u[:])

                s8 = ep_pool.tile([P, g], F32, tag="s8")
                nc.vector.tensor_reduce(
                    s8[:], qun[:], axis=mybir.AxisListType.X,
                    op=mybir.AluOpType.add)
                r8 = ep_pool.tile([P, g], F32, tag="r8")
                nc.vector.reciprocal(r8[:], s8[:])

                qout = qo_pool.tile([P, g, K], BF16 if qo_bf16 else F32)
                nc.vector.tensor_tensor(
                    out=qout[:], in0=qun[:], in1=_bcast_free(r8[:], K),
                    op=mybir.AluOpType.mult)

                q_view = q_dram[n0:n0 + P * g, :].rearrange(
                    "(gg p) k -> p gg k", p=P)
                nc.sync.dma_start(q_view, qout[:])

    nc.compile()
    return nc


_CACHE = {}


def _get_nc():
    if "nc" not in _CACHE:
        _CACHE["nc"] = build()
    return _CACHE["nc"]


def kernel(x: np.ndarray, clusters: np.ndarray) -> np.ndarray:
    from concourse.bass_utils import run_bass_kernel_spmd

    x = np.ascontiguousarray(x, dtype=np.float32)
    clusters = np.ascontiguousarray(clusters, dtype=np.float32)
    nc = _get_nc()
    in_maps = [
        {"x": x[i * NS:(i + 1) * NS], "clusters": clusters}
        for i in range(N_CORES)
    ]
    res = run_bass_kernel_spmd(nc, in_maps, core_ids=list(range(N_CORES)))
    out = np.concatenate([r["q"] for r in res.results], axis=0)
    return np.ascontiguousarray(out.astype(np.float32))


# revision 3
# speedup vs baseline: 1.1333x; 1.1333x over previous
# Trainium2 Bass kernel for nn_ClusteringLayer (DEC soft-assignment / Student-t
# codebook posterior):
#   d2[n,k] = ||x_n - c_k||^2 ;  q = 1/(1+d2) row-normalized over k  (alpha=1).
#
# Sharding: data-parallel along N over 8 NeuronCores; clusters replicated.
# Per core: x_shard (16384, 512) f32 -> q_shard (16384, 128) f32.
#
# Engine balance per 1024-row super-tile (measured-driven; DVE is the
# critical engine, so work is spread):
#   Pool/SWDGE: cast-load x f32->bf16 (software-pipelined one tile ahead)
#   ACT:        per-tile Square + fp32 accum -> x2
#   PE:         4 transposes/tile -> PSUM; 4 bf16 MMs + rank-1 (1+c2) row
#   DVE:        PSUM->SBUF xT copies (batched FD=2048), u = psum + x2,
#               reciprocal, row-sum, tiny reciprocal, normalize multiply
#   SP/HWDGE:   store q
import dataclasses

import numpy as np

import concourse.bass as bass
import concourse.mybir as mybir
from concourse import bacc
from concourse.bass import ts
from concourse.masks import make_identity
from concourse.tile import TileContext

N, D, K = 131072, 512, 128
N_CORES = 8
NS = N // N_CORES  # rows per core
P = 128  # partitions / row-tile size
G = 8    # row-tiles per super-tile
F32 = mybir.dt.float32
BF16 = mybir.dt.bfloat16


def _bcast_free(ap: bass.AP, n: int) -> bass.AP:
    """Append a step-0 (broadcast) innermost free dim of size n."""
    return dataclasses.replace(ap, ap=list(ap.ap) + [[0, n]])


def build(ns=NS, g=G, repeat=1, dma_mode="normal", xin_bufs=3, xt_bufs=3,
          ep_bufs=3, qo_bufs=3, ps_t_bufs=2, ps_q_bufs=2, qo_bf16=0):
    n_super = ns // (P * g)
    assert ns == n_super * P * g
    n_dchunk = D // P  # 4
    half_g = g // 2

    nc = bacc.Bacc("TRN2", target_bir_lowering=False, debug=False)
    x_dram = nc.dram_tensor("x", [ns, D], F32, kind="ExternalInput")
    c_dram = nc.dram_tensor("clusters", [K, D], F32, kind="ExternalInput")
    q_dram = nc.dram_tensor("q", [ns, K], BF16 if qo_bf16 else F32,
                            kind="ExternalOutput")

    with TileContext(nc) as tc:
        with (
            tc.tile_pool(name="const", bufs=1) as const_pool,
            tc.tile_pool(name="xin", bufs=xin_bufs) as xin_pool,
            tc.tile_pool(name="xt", bufs=xt_bufs) as xt_pool,
            tc.tile_pool(name="ep", bufs=ep_bufs) as ep_pool,
            tc.tile_pool(name="qo", bufs=qo_bufs) as qo_pool,
            tc.tile_pool(name="ps_t", bufs=ps_t_bufs, space="PSUM") as ps_t_pool,
            tc.tile_pool(name="ps_q", bufs=ps_q_bufs, space="PSUM") as ps_q_pool,
        ):
            # ---------------- setup (once) ----------------
            ident_bf = const_pool.tile([P, P], BF16)
            make_identity(nc, ident_bf)

            c_f32 = const_pool.tile([K, D], F32)
            nc.sync.dma_start(c_f32[:], c_dram[:, :])
            c_bf = const_pool.tile([K, D], BF16)
            nc.vector.tensor_copy(c_bf[:], c_f32[:])

            # c2[k] = sum_d c_bf[k,d]^2 (fp32 accum), then 1 + c2 as bf16
            csq = const_pool.tile([K, D], F32)
            c2 = const_pool.tile([K, 1], F32)
            nc.scalar.activation(
                csq[:], c_bf[:], mybir.ActivationFunctionType.Square,
                accum_out=c2[:],
            )
            c2p1_bf = const_pool.tile([K, 1], BF16)
            nc.vector.tensor_scalar_add(c2p1_bf[:], c2[:], 1.0)

            # transpose (1+c2) -> row [1, K] bf16
            ps_row = ps_t_pool.tile([1, K], BF16, tag="ps_xt")
            nc.tensor.transpose(ps_row[:], c2p1_bf[:], ident_bf[:])
            c2p1_row = const_pool.tile([1, K], BF16)
            nc.vector.tensor_copy(c2p1_row[:], ps_row[:])

            ones_row = const_pool.tile([1, K], BF16)
            nc.vector.memset(ones_row[:], 1.0)

            # cTm2[p, c, k] = -2 * clusters_bf[k, c*128+p]
            cTm2 = const_pool.tile([P, n_dchunk, K], BF16)
            for c in range(n_dchunk):
                ps_c = ps_t_pool.tile([P, P], BF16, tag="ps_xt")
                nc.tensor.transpose(ps_c[:], c_bf[:, ts(c, P)], ident_bf[:])
                nc.vector.tensor_scalar_mul(cTm2[:, c, :], ps_c[:], -2.0)

            # ---------------- main loop ----------------
            # x loads software-pipelined: st+1's SWDGE cast-load issued at
            # the start of st's body.
            def issue_load(sti):
                n0 = (sti % n_super) * P * g
                x_view = x_dram[n0:n0 + P * g, :].rearrange(
                    "(gg p) d -> p gg d", p=P)
                t = xin_pool.tile([P, g, D], BF16, name="x_nat", tag="x_nat")
                nc.gpsimd.dma_start(t[:], x_view)
                return t

            n_total = n_super * repeat
            if dma_mode == "once":
                x_once = issue_load(0)
            else:
                pending = issue_load(0)
            for sti in range(n_total):
                st = sti % n_super
                n0 = st * P * g
                if dma_mode == "once":
                    x_nat = x_once
                else:
                    x_nat = pending
                    if sti + 1 < n_total:
                        pending = issue_load(sti + 1)

                x2s = ep_pool.tile([P, g], F32, tag="x2s")
                psum_q = ps_q_pool.tile([P, g, K], F32)

                # squares (ACT) + transposes (PE); copies batched per 4 tiles
                for gp in range(g // half_g):  # 2 groups of 4 row-tiles
                    ps_xt = ps_t_pool.tile(
                        [P, half_g * n_dchunk, P], BF16, tag="ps_xt")
                    for gi in range(half_g):
                        gg = gp * half_g + gi
                        sq_scr = xt_pool.tile([P, D], BF16, tag="sq")
                        nc.scalar.activation(
                            sq_scr[:], x_nat[:, gg, :],
                            mybir.ActivationFunctionType.Square,
                            accum_out=x2s[:, gg:gg + 1],
                        )
                        for c in range(n_dchunk):
                            nc.tensor.transpose(
                                ps_xt[:, gi * n_dchunk + c, :],
                                x_nat[:, gg, ts(c, P)], ident_bf[:])
                    xt2 = xt_pool.tile(
                        [P, half_g * n_dchunk, P], BF16, tag=f"xt{gp}")
                    # split the PSUM->SBUF xT copies across DVE and ACT:
                    # DVE is the critical engine; ACT only runs the squares.
                    if gp % 2 == 1:
                        nc.scalar.copy(xt2[:], ps_xt[:])
                    else:
                        nc.vector.tensor_copy(xt2[:], ps_xt[:])

                    for gi in range(half_g):
                        gg = gp * half_g + gi
                        for c in range(n_dchunk):
                            nc.tensor.matmul(
                                psum_q[:, gg, :],
                                lhsT=xt2[:, gi * n_dchunk + c, :],
                                rhs=cTm2[:, c, :], start=(c == 0), stop=False)
                        nc.tensor.matmul(
                            psum_q[:, gg, :], lhsT=ones_row[:],
                            rhs=c2p1_row[:], start=False, stop=True)

                # u = psum + x2[n] broadcast along k (clamp at 1.0 skipped:
                # d2 >= -1e-4 numerically; ref's max(d2,0) differs <=1e-5)
                u = ep_pool.tile([P, g, K], F32, tag="u")
                nc.vector.tensor_tensor(
                    out=u[:], in0=psum_q[:],
                    in1=_bcast_free(x2s[:], K),
                    op=mybir.AluOpType.add,
                )
                qun = ep_pool.tile([P, g, K], F32, tag="qun")
                nc.vector.reciprocal(qun[:], u[:])

                s8 = ep_pool.tile([P, g], F32, tag="s8")
                nc.vector.tensor_reduce(
                    s8[:], qun[:], axis=mybir.AxisListType.X,
                    op=mybir.AluOpType.add)
                r8 = ep_pool.tile([P, g], F32, tag="r8")
                nc.vector.reciprocal(r8[:], s8[:])

                qout = qo_pool.tile([P, g, K], BF16 if qo_bf16 else F32)
                nc.vector.tensor_tensor(
                    out=qout[:], in0=qun[:], in1=_bcast_free(r8[:], K),
                    op=mybir.AluOpType.mult)

                q_view = q_dram[n0:n0 + P * g, :].rearrange(
                    "(gg p) k -> p gg k", p=P)
                nc.sync.dma_start(q_view, qout[:])

    nc.compile()
    return nc


_CACHE = {}


def _get_nc():
    if "nc" not in _CACHE:
        _CACHE["nc"] = build()
    return _CACHE["nc"]


def kernel(x: np.ndarray, clusters: np.ndarray) -> np.ndarray:
    from concourse.bass_utils import run_bass_kernel_spmd

    x = np.ascontiguousarray(x, dtype=np.float32)
    clusters = np.ascontiguousarray(clusters, dtype=np.float32)
    nc = _get_nc()
    in_maps = [
        {"x": x[i * NS:(i + 1) * NS], "clusters": clusters}
        for i in range(N_CORES)
    ]
    res = run_bass_kernel_spmd(nc, in_maps, core_ids=list(range(N_CORES)))
    out = np.concatenate([r["q"] for r in res.results], axis=0)
    return np.ascontiguousarray(out.astype(np.float32))


# revision 4
# speedup vs baseline: 1.4571x; 1.2857x over previous
# Trainium2 Bass kernel for nn_ClusteringLayer (DEC soft-assignment / Student-t
# codebook posterior):
#   d2[n,k] = ||x_n - c_k||^2 ;  q = 1/(1+d2) row-normalized over k  (alpha=1).
#
# Sharding: data-parallel along N over 8 NeuronCores; clusters replicated.
# Per core: x_shard (16384, 512) f32 -> q_shard (16384, 128) f32.
#
# Engine balance per 1024-row super-tile (measured-driven; DVE is the
# critical engine, so work is spread):
#   Pool/SWDGE: cast-load x f32->bf16 (software-pipelined one tile ahead)
#   ACT:        per-tile Square + fp32 accum -> x2
#   PE:         4 transposes/tile -> PSUM; 4 bf16 MMs + rank-1 (1+c2) row
#   DVE:        PSUM->SBUF xT copies (batched FD=2048), u = psum + x2,
#               reciprocal, row-sum, tiny reciprocal, normalize multiply
#   SP/HWDGE:   store q
import dataclasses

import numpy as np

import concourse.bass as bass
import concourse.mybir as mybir
from concourse import bacc
from concourse.bass import ts
from concourse.masks import make_identity
from concourse.tile import TileContext

N, D, K = 131072, 512, 128
N_CORES = 8
NS = N // N_CORES  # rows per core
P = 128  # partitions / row-tile size
G = 8    # row-tiles per super-tile
F32 = mybir.dt.float32
BF16 = mybir.dt.bfloat16


def _bcast_free(ap: bass.AP, n: int) -> bass.AP:
    """Append a step-0 (broadcast) innermost free dim of size n."""
    return dataclasses.replace(ap, ap=list(ap.ap) + [[0, n]])


def build(ns=NS, g=G, repeat=1, dma_mode="normal", xin_bufs=3, xt_bufs=3,
          ep_bufs=3, qo_bufs=3, ps_t_bufs=2, ps_q_bufs=2, qo_bf16=0):
    n_super = ns // (P * g)
    assert ns == n_super * P * g
    n_dchunk = D // P  # 4
    half_g = g // 2

    nc = bacc.Bacc("TRN2", target_bir_lowering=False, debug=False)
    x_dram = nc.dram_tensor("x", [ns, D], F32, kind="ExternalInput")
    c_dram = nc.dram_tensor("clusters", [K, D], F32, kind="ExternalInput")
    q_dram = nc.dram_tensor("q", [ns, K], BF16 if qo_bf16 else F32,
                            kind="ExternalOutput")

    with TileContext(nc) as tc:
        with (
            tc.tile_pool(name="const", bufs=1) as const_pool,
            tc.tile_pool(name="xin", bufs=xin_bufs) as xin_pool,
            tc.tile_pool(name="xt", bufs=xt_bufs) as xt_pool,
            tc.tile_pool(name="ep", bufs=ep_bufs) as ep_pool,
            tc.tile_pool(name="qo", bufs=qo_bufs) as qo_pool,
            tc.tile_pool(name="ps_t", bufs=ps_t_bufs, space="PSUM") as ps_t_pool,
            tc.tile_pool(name="ps_q", bufs=ps_q_bufs, space="PSUM") as ps_q_pool,
        ):
            # ---------------- setup (once) ----------------
            ident_bf = const_pool.tile([P, P], BF16)
            make_identity(nc, ident_bf)

            c_f32 = const_pool.tile([K, D], F32)
            nc.sync.dma_start(c_f32[:], c_dram[:, :])
            c_bf = const_pool.tile([K, D], BF16)
            nc.vector.tensor_copy(c_bf[:], c_f32[:])

            # c2[k] = sum_d c_bf[k,d]^2 (fp32 accum), then 1 + c2 as bf16
            csq = const_pool.tile([K, D], F32)
            c2 = const_pool.tile([K, 1], F32)
            nc.scalar.activation(
                csq[:], c_bf[:], mybir.ActivationFunctionType.Square,
                accum_out=c2[:],
            )
            c2p1_bf = const_pool.tile([K, 1], BF16)
            nc.vector.tensor_scalar_add(c2p1_bf[:], c2[:], 1.0)

            # transpose (1+c2) -> row [1, K] bf16
            ps_row = ps_t_pool.tile([1, K], BF16, tag="ps_xt")
            nc.tensor.transpose(ps_row[:], c2p1_bf[:], ident_bf[:])
            c2p1_row = const_pool.tile([1, K], BF16)
            nc.vector.tensor_copy(c2p1_row[:], ps_row[:])

            ones_row = const_pool.tile([1, K], BF16)
            nc.vector.memset(ones_row[:], 1.0)

            # cTm2[p, c, k] = -2 * clusters_bf[k, c*128+p]
            cTm2 = const_pool.tile([P, n_dchunk, K], BF16)
            for c in range(n_dchunk):
                ps_c = ps_t_pool.tile([P, P], BF16, tag="ps_xt")
                nc.tensor.transpose(ps_c[:], c_bf[:, ts(c, P)], ident_bf[:])
                nc.vector.tensor_scalar_mul(cTm2[:, c, :], ps_c[:], -2.0)

            # ---------------- main loop ----------------
            # x loads software-pipelined: st+1's SWDGE cast-load issued at
            # the start of st's body.
            def issue_load(sti):
                n0 = (sti % n_super) * P * g
                x_view = x_dram[n0:n0 + P * g, :].rearrange(
                    "(gg p) d -> p gg d", p=P)
                t = xin_pool.tile([P, g, D], BF16, name="x_nat", tag="x_nat")
                nc.gpsimd.dma_start(t[:], x_view)
                return t

            n_total = n_super * repeat
            if dma_mode == "once":
                x_once = issue_load(0)
            else:
                pending = issue_load(0)
            for sti in range(n_total):
                st = sti % n_super
                n0 = st * P * g
                if dma_mode == "once":
                    x_nat = x_once
                else:
                    x_nat = pending
                    if sti + 1 < n_total:
                        pending = issue_load(sti + 1)

                x2s = ep_pool.tile([P, g], F32, tag="x2s")
                psum_q = ps_q_pool.tile([P, g, K], F32)

                # squares (ACT) + transposes (PE); copies batched per 4 tiles
                for gp in range(g // half_g):  # 2 groups of 4 row-tiles
                    ps_xt = ps_t_pool.tile(
                        [P, half_g * n_dchunk, P], BF16, tag="ps_xt")
                    for gi in range(half_g):
                        gg = gp * half_g + gi
                        sq_scr = xt_pool.tile([P, D], BF16, tag="sq")
                        nc.scalar.activation(
                            sq_scr[:], x_nat[:, gg, :],
                            mybir.ActivationFunctionType.Square,
                            accum_out=x2s[:, gg:gg + 1],
                        )
                        for c in range(n_dchunk):
                            nc.tensor.transpose(
                                ps_xt[:, gi * n_dchunk + c, :],
                                x_nat[:, gg, ts(c, P)], ident_bf[:])
                    xt2 = xt_pool.tile(
                        [P, half_g * n_dchunk, P], BF16, tag=f"xt{gp}")
                    # PSUM->SBUF xT copies on ACT: DVE is the critical
                    # engine (u-add, reciprocals, row-sum, normalize mult);
                    # ACT only runs the squares, so it absorbs the copies.
                    # Measured: 104.7us (ACT) vs 134.7 (split) vs 158.5
                    # (DVE) per-iteration.
                    nc.scalar.copy(xt2[:], ps_xt[:])

                    for gi in range(half_g):
                        gg = gp * half_g + gi
                        for c in range(n_dchunk):
                            nc.tensor.matmul(
                                psum_q[:, gg, :],
                                lhsT=xt2[:, gi * n_dchunk + c, :],
                                rhs=cTm2[:, c, :], start=(c == 0), stop=False)
                        nc.tensor.matmul(
                            psum_q[:, gg, :], lhsT=ones_row[:],
                            rhs=c2p1_row[:], start=False, stop=True)

                # u = psum + x2[n] broadcast along k (clamp at 1.0 skipped:
                # d2 >= -1e-4 numerically; ref's max(d2,0) differs <=1e-5)
                u = ep_pool.tile([P, g, K], F32, tag="u")
                nc.vector.tensor_tensor(
                    out=u[:], in0=psum_q[:],
                    in1=_bcast_free(x2s[:], K),
                    op=mybir.AluOpType.add,
                )
                qun = ep_pool.tile([P, g, K], F32, tag="qun")
                nc.vector.reciprocal(qun[:], u[:])

                s8 = ep_pool.tile([P, g], F32, tag="s8")
                nc.vector.tensor_reduce(
                    s8[:], qun[:], axis=mybir.AxisListType.X,
                    op=mybir.AluOpType.add)
                r8 = ep_pool.tile([P, g], F32, tag="r8")
                nc.vector.reciprocal(r8[:], s8[:])

                qout = qo_pool.tile([P, g, K], BF16 if qo_bf16 else F32)
                nc.vector.tensor_tensor(
                    out=qout[:], in0=qun[:], in1=_bcast_free(r8[:], K),
                    op=mybir.AluOpType.mult)

                q_view = q_dram[n0:n0 + P * g, :].rearrange(
                    "(gg p) k -> p gg k", p=P)
                nc.sync.dma_start(q_view, qout[:])

    nc.compile()
    return nc


_CACHE = {}


def _get_nc():
    if "nc" not in _CACHE:
        _CACHE["nc"] = build()
    return _CACHE["nc"]


def kernel(x: np.ndarray, clusters: np.ndarray) -> np.ndarray:
    from concourse.bass_utils import run_bass_kernel_spmd

    x = np.ascontiguousarray(x, dtype=np.float32)
    clusters = np.ascontiguousarray(clusters, dtype=np.float32)
    nc = _get_nc()
    in_maps = [
        {"x": x[i * NS:(i + 1) * NS], "clusters": clusters}
        for i in range(N_CORES)
    ]
    res = run_bass_kernel_spmd(nc, in_maps, core_ids=list(range(N_CORES)))
    out = np.concatenate([r["q"] for r in res.results], axis=0)
    return np.ascontiguousarray(out.astype(np.float32))
